# revision 1
# baseline (speedup 1.0000x reference)
"""3-layer LSTM decoder + projection + softmax on 8 trn2 NeuronCores.

Sharding: hidden units / gates sharded 8 ways (512 gates = [i|f|o|g] x 128
units per core). Each wavefront step computes L1(s), L2(s-1), L3(s-2),
projection(s-3); one AllGather per step exchanges the three transposed
h-slices [128u, 64b] x 3 layers. Recurrent matmuls keep h^T stationary and
stream the fp32r weight slices resident in SBUF. Projection is replicated
(full V=512) per core; each core writes its T/8 time-window of the outputs.
"""
import numpy as np

import concourse.bass as bass
import concourse.bacc as bacc
import concourse.mybir as mybir
from concourse.tile import TileContext
from concourse.bass_utils import run_bass_kernel_spmd

F32 = mybir.dt.float32
F32R = mybir.dt.float32r
ACT = mybir.ActivationFunctionType
ALU = mybir.AluOpType

B, T, U, V, L, F = 64, 256, 1024, 512, 512, 64
NCORE = 8
GS = 512          # gates per core (4 x 128 units)
US = 128          # units per core
KT = U // 128     # 8 contract tiles over hidden
TWIN = T // NCORE


def build(t_steps=T):
    nc = bacc.Bacc("TRN2", target_bir_lowering=False)
    TT = t_steps

    # ---- external inputs (per-core prepped content, same shapes everywhere)
    latT = nc.dram_tensor("latT", [L, B], F32R, kind="ExternalInput")
    xT = nc.dram_tensor("xT", [F, TT, B], F32R, kind="ExternalInput")
    k1L = nc.dram_tensor("k1L", [L, GS], F32R, kind="ExternalInput")
    k1x = nc.dram_tensor("k1x", [F, GS], F32R, kind="ExternalInput")
    rk1 = nc.dram_tensor("rk1", [U, GS], F32R, kind="ExternalInput")
    k2 = nc.dram_tensor("k2", [U, GS], F32R, kind="ExternalInput")
    rk2 = nc.dram_tensor("rk2", [U, GS], F32R, kind="ExternalInput")
    k3 = nc.dram_tensor("k3", [U, GS], F32R, kind="ExternalInput")
    rk3 = nc.dram_tensor("rk3", [U, GS], F32R, kind="ExternalInput")
    wp = nc.dram_tensor("wp", [U, V], F32R, kind="ExternalInput")
    b1r = nc.dram_tensor("b1r", [1, GS], F32R, kind="ExternalInput")
    b2r = nc.dram_tensor("b2r", [1, GS], F32R, kind="ExternalInput")
    b3r = nc.dram_tensor("b3r", [1, GS], F32R, kind="ExternalInput")
    bvr = nc.dram_tensor("bvr", [1, V], F32R, kind="ExternalInput")
    eye64 = nc.dram_tensor("eye64", [B, B], F32R, kind="ExternalInput")
    eye128 = nc.dram_tensor("eye128", [128, 128], F32R, kind="ExternalInput")
    ones1 = nc.dram_tensor("ones1", [1, B], F32R, kind="ExternalInput")

    y_out = nc.dram_tensor("y_out", [B, TWIN if TT == T else TT, V], F32,
                           kind="ExternalOutput")
    l_out = nc.dram_tensor("l_out", [B, TWIN if TT == T else TT, V], F32,
                           kind="ExternalOutput")

    # ---- internal DRAM
    ydram = nc.dram_tensor("ydram", [B, TT, V], F32)
    ldram = nc.dram_tensor("ldram", [B, TT, V], F32)
    din = [nc.dram_tensor(f"din{p}", [128, 3, B], F32R) for p in range(2)]
    dout = [nc.dram_tensor(f"dout{p}", [U, 3, B], F32R, addr_space="Shared")
            for p in range(2)]
    RG = [list(range(NCORE))]

    XCH = 32  # x chunk steps
    n_xch = (TT + XCH - 1) // XCH

    with TileContext(nc) as tc:
        with (
            tc.tile_pool(name="wpool", bufs=1) as wpool,
            tc.tile_pool(name="state", bufs=1) as state,
            tc.tile_pool(name="work", bufs=2) as work,
            tc.tile_pool(name="ps", bufs=1, space="PSUM") as ps,
            tc.tile_pool(name="pstr", bufs=2, space="PSUM") as pstr,
        ):
            # ---- resident weights
            def load_w(name, dram, kt, ncol):
                t = wpool.tile([128, kt, ncol], F32R, name=name, tag=name)
                nc.sync.dma_start(
                    t[:, :, :], dram.ap().rearrange("(k p) g -> p k g", p=128))
                return t

            rk1s = load_w("rk1s", rk1, KT, GS)
            rk2s = load_w("rk2s", rk2, KT, GS)
            rk3s = load_w("rk3s", rk3, KT, GS)
            k2s = load_w("k2s", k2, KT, GS)
            k3s = load_w("k3s", k3, KT, GS)
            ws = load_w("ws", wp, KT, V)
            k1Ls = load_w("k1Ls", k1L, L // 128, GS)
            latTs = load_w("latTs", latT, L // 128, B)
            k1xs = wpool.tile([F, GS], F32R, tag="k1xs")
            nc.sync.dma_start(k1xs[:, :], k1x[:, :])
            e64 = wpool.tile([B, B], F32R, tag="e64")
            nc.sync.dma_start(e64[:, :], eye64[:, :])
            e128 = wpool.tile([128, 128], F32R, tag="e128")
            nc.sync.dma_start(e128[:, :], eye128[:, :])
            on1 = wpool.tile([1, B], F32R, tag="on1")
            nc.sync.dma_start(on1[:, :], ones1[:, :])
            b2s = wpool.tile([1, GS], F32R, tag="b2s")
            nc.sync.dma_start(b2s[:, :], b2r[:, :])
            b3s = wpool.tile([1, GS], F32R, tag="b3s")
            nc.sync.dma_start(b3s[:, :], b3r[:, :])
            bvs = wpool.tile([1, V], F32R, tag="bvs")
            nc.sync.dma_start(bvs[:, :], bvr[:, :])
            b1s = wpool.tile([1, GS], F32R, tag="b1s")
            nc.sync.dma_start(b1s[:, :], b1r[:, :])

            # ---- latentb = latT.T @ k1L + b1  (once)
            lb_ps = pstr.tile([B, GS], F32, tag="lbps")
            for k in range(L // 128):
                nc.tensor.matmul(lb_ps[:, :], latTs[:, k, :], k1Ls[:, k, :],
                                 start=(k == 0), stop=False)
            nc.tensor.matmul(lb_ps[:, :], on1[:, :], b1s[:, :],
                             start=False, stop=True)
            latentb = state.tile([B, GS], F32R, tag="latentb")
            nc.vector.tensor_copy(latentb[:, :], lb_ps[:, :])

            # ---- states
            cst = [state.tile([B, US], F32, name=f"c{l}", tag=f"c{l}")
                   for l in range(3)]
            for c in cst:
                nc.vector.memset(c[:, :], 0.0)
            h_all = [state.tile([128, 3 * KT * B], F32R, name=f"hall{p}",
                                tag=f"hall{p}") for p in range(2)]
            for h in h_all:
                nc.vector.memset(h[:, :].bitcast(F32), 0.0)
            evac = [state.tile([128, 3 * B], F32R, name=f"evac{p}",
                               tag=f"evac{p}") for p in range(2)]
            for e in evac:
                nc.vector.memset(e[:, :].bitcast(F32), 0.0)

            def hp_slice(hbuf, layer, k):
                off = (layer * KT + k) * B
                return hbuf[:, off:off + B]

            xsb = [state.tile([F, XCH, B], F32R, name=f"xsb{p}", tag=f"xsb{p}")
                   for p in range(2)]

            gates_sb = [work.tile([B, GS], F32, name=f"gsb{l}", tag=f"gsb{l}")
                        for l in range(3)]
            gps = [ps.tile([B, GS], F32, name=f"gps{l}", tag=f"gps{l}")
                   for l in range(3)]
            pps = ps.tile([B, V], F32, tag="pps")

            def cell(layer, hp, t, par, rks, ins_fn):
                """One LSTM cell step for `layer` at time t. ins_fn adds the
                input contributions (first matmul must set start=True).
                Returns nothing; writes evac slice."""
                g = gps[layer]
                ins_fn(g)
                for k in range(KT):
                    nc.tensor.matmul(g[:, :], hp_slice(hp, layer, k), rks[:, k, :],
                                     start=False, stop=(k == KT - 1))
                gs = gates_sb[layer]
                nc.scalar.activation(gs[:, 0:384], g[:, 0:384], ACT.Sigmoid)
                nc.scalar.activation(gs[:, 384:512], g[:, 384:512], ACT.Tanh)
                c = cst[layer]
                tmp = work.tile([B, US], F32, name=f"tmp{layer}_{t}",
                                tag=f"tmp{layer}")
                nc.vector.tensor_tensor(tmp[:, :], gs[:, 0:128], gs[:, 384:512],
                                        ALU.mult)
                nc.vector.tensor_tensor(c[:, :], gs[:, 128:256], c[:, :],
                                        ALU.mult)
                nc.vector.tensor_tensor(c[:, :], c[:, :], tmp[:, :], ALU.add)
                th = work.tile([B, US], F32, name=f"th{layer}_{t}",
                               tag=f"th{layer}")
                nc.scalar.activation(th[:, :], c[:, :], ACT.Tanh)
                hsb = work.tile([B, US], F32R, name=f"h{layer}_{t}",
                                tag=f"h{layer}")
                nc.vector.tensor_tensor(hsb[:, :], gs[:, 256:384], th[:, :],
                                        ALU.mult)
                # transpose -> evac slot
                tp = pstr.tile([128, B], F32R, name=f"tp{layer}_{t}",
                               tag="trps")
                nc.tensor.transpose(tp[:, :], hsb[:, :], e64[:, :])
                nc.vector.tensor_copy(evac[par][:, layer * B:(layer + 1) * B], tp[:, :])

            nsteps = TT + 3
            for s in range(nsteps):
                par = s % 2
                hp = h_all[(s + 1) % 2]  # written by AG(s-1)
                t1, t2, t3, tpj = s, s - 1, s - 2, s - 3

                # L1
                if 0 <= t1 < TT:
                    if t1 % XCH == 0:
                        xb = xsb[(t1 // XCH) % 2]
                        hi = min(XCH, TT - t1)
                        nc.sync.dma_start(xb[:, 0:hi, :], xT[:, t1:t1 + hi, :])

                    def ins1(g, t1=t1):
                        xb = xsb[(t1 // XCH) % 2]
                        nc.tensor.matmul(g[:, :], xb[:, t1 % XCH, :],
                                         k1xs[:, :], start=True, stop=False)
                        nc.tensor.matmul(g[:, :], e64[:, :], latentb[:, :],
                                         start=False, stop=False)
                    cell(0, hp, t1, par, rk1s, ins1)

                # L2
                if 0 <= t2 < TT:
                    def ins2(g):
                        nc.tensor.matmul(g[:, :], on1[:, :], b2s[:, :],
                                         start=True, stop=False)
                        for k in range(KT):
                            nc.tensor.matmul(g[:, :], hp_slice(hp, 0, k),
                                             k2s[:, k, :], start=False,
                                             stop=False)
                    cell(1, hp, t2, par, rk2s, ins2)

                # L3
                if 0 <= t3 < TT:
                    def ins3(g):
                        nc.tensor.matmul(g[:, :], on1[:, :], b3s[:, :],
                                         start=True, stop=False)
                        for k in range(KT):
                            nc.tensor.matmul(g[:, :], hp_slice(hp, 1, k),
                                             k3s[:, k, :], start=False,
                                             stop=False)
                    cell(2, hp, t3, par, rk3s, ins3)

                # projection + softmax for step tpj
                if 0 <= tpj < TT:
                    nc.tensor.matmul(pps[:, :], on1[:, :], bvs[:, :],
                                     start=True, stop=False)
                    for k in range(KT):
                        nc.tensor.matmul(pps[:, :], hp_slice(hp, 2, k),
                                         ws[:, k, :], start=False,
                                         stop=(k == KT - 1))
                    lsb = work.tile([B, V], F32, name=f"lsb{tpj}", tag="lsb")
                    nc.vector.tensor_copy(lsb[:, :], pps[:, :])
                    nmx = work.tile([B, 1], F32, name=f"nmx{tpj}", tag="nmx")
                    nc.vector.tensor_reduce(nmx[:, :], lsb[:, :],
                                            axis=mybir.AxisListType.X,
                                            op=ALU.max, negate=True)
                    esb = work.tile([B, V], F32, name=f"esb{tpj}", tag="esb")
                    nc.scalar.activation(esb[:, :], lsb[:, :], ACT.Exp,
                                         bias=nmx[:, 0:1])
                    sm = work.tile([B, 1], F32, name=f"sm{tpj}", tag="sm")
                    nc.vector.tensor_reduce(sm[:, :], esb[:, :],
                                            axis=mybir.AxisListType.X,
                                            op=ALU.add)
                    rs = work.tile([B, 1], F32, name=f"rs{tpj}", tag="rs")
                    nc.vector.reciprocal(rs[:, :], sm[:, :])
                    ysb = work.tile([B, V], F32, name=f"ysb{tpj}", tag="ysb")
                    nc.vector.tensor_scalar(ysb[:, :], esb[:, :], rs[:, 0:1],
                                            None, ALU.mult)
                    nc.sync.dma_start(ydram[:, tpj:tpj + 1, :], ysb[:, :])
                    nc.sync.dma_start(ldram[:, tpj:tpj + 1, :], lsb[:, :])

                # combined AllGather of the three h-slices
                if s < TT + 2:
                    nc.sync.dma_start(
                        din[par].ap().rearrange("p l b -> p (l b)"),
                        evac[par][:, :])
                    nc.gpsimd.collective_compute(
                        "AllGather", ALU.bypass, replica_groups=RG,
                        ins=[din[par].ap().opt()],
                        outs=[dout[par].ap().opt()],
                    )
                    nc.sync.dma_start(
                        h_all[par][:, :].rearrange("p (l k b) -> p l k b",
                                                   l=3, k=KT),
                        dout[par].ap().rearrange("(k p) l b -> p l k b", p=128),
                    )

            # windowed output copy (per-core time window)
            if TT == T:
                pid = nc.gpsimd.partition_id()
                off = pid * TWIN
                nc.gpsimd.dma_start(y_out[:, :, :],
                                    ydram[:, bass.ds(off, TWIN), :])
                nc.gpsimd.dma_start(l_out[:, :, :],
                                    ldram[:, bass.ds(off, TWIN), :])
            else:
                nc.gpsimd.dma_start(y_out[:, :, :], ydram[:, :, :])
                nc.gpsimd.dma_start(l_out[:, :, :], ldram[:, :, :])

    nc.compile()
    return nc


_built = {}


def _get_nc(t_steps):
    if t_steps not in _built:
        _built[t_steps] = build(t_steps)
    return _built[t_steps]


def _prep_inputs(latent, x, k1, rk1, b1, k2, rk2, b2, k3, rk3, b3, w, b,
                 t_steps):
    f32 = np.float32
    latent = np.asarray(latent, f32)
    x = np.asarray(x, f32)
    in_maps = []
    latT = np.ascontiguousarray(latent.T)                   # [L, B]
    xT = np.ascontiguousarray(np.transpose(x, (2, 1, 0)))   # [F, T, B]
    eye64v = np.eye(B, dtype=f32)
    eye128v = np.eye(128, dtype=f32)
    ones1v = np.ones((1, B), f32)
    for j in range(NCORE):
        u0 = j * US
        cols = np.concatenate([
            np.arange(u0, u0 + US),                  # i
            np.arange(U + u0, U + u0 + US),          # f
            np.arange(3 * U + u0, 3 * U + u0 + US),  # o
            np.arange(2 * U + u0, 2 * U + u0 + US),  # g
        ])
        in_maps.append({
            "latT": latT,
            "xT": np.ascontiguousarray(xT[:, :t_steps, :]),
            "k1L": np.ascontiguousarray(k1[:L, cols]),
            "k1x": np.ascontiguousarray(k1[L:, cols]),
            "rk1": np.ascontiguousarray(rk1[:, cols]),
            "k2": np.ascontiguousarray(k2[:, cols]),
            "rk2": np.ascontiguousarray(rk2[:, cols]),
            "k3": np.ascontiguousarray(k3[:, cols]),
            "rk3": np.ascontiguousarray(rk3[:, cols]),
            "wp": np.ascontiguousarray(np.asarray(w, f32)),
            "b1r": np.ascontiguousarray(np.asarray(b1, f32)[cols][None, :]),
            "b2r": np.ascontiguousarray(np.asarray(b2, f32)[cols][None, :]),
            "b3r": np.ascontiguousarray(np.asarray(b3, f32)[cols][None, :]),
            "bvr": np.ascontiguousarray(np.asarray(b, f32)[None, :]),
            "eye64": eye64v,
            "eye128": eye128v,
            "ones1": ones1v,
        })
    return in_maps


def run(t_steps=T, **inputs):
    nc = _get_nc(t_steps)
    in_maps = _prep_inputs(t_steps=t_steps, **inputs)
    res = run_bass_kernel_spmd(nc, in_maps, core_ids=list(range(NCORE)))
    if t_steps == T:
        y = np.concatenate([res.results[j]["y_out"] for j in range(NCORE)],
                           axis=1)
        lg = np.concatenate([res.results[j]["l_out"] for j in range(NCORE)],
                            axis=1)
    else:
        y = res.results[0]["y_out"]
        lg = res.results[0]["l_out"]
    return y, lg


def kernel(**inputs):
    return run(t_steps=T, **inputs)



# revision 2
# speedup vs baseline: 1.0250x; 1.0250x over previous
"""3-layer LSTM decoder + projection + softmax on 8 trn2 NeuronCores. v2.

Sharding: gates sharded 8 ways (512 of 4096 gate cols per core, blocks
[i|f|o|g] x 128 units). Wavefront: step s computes L1(s), L2(s-1),
L3(s-2). Changes vs v1:
  - fp16 h-path: h slices, AllGather payloads, and all h-consuming
    weights (rk*, k2, k3, w) are fp16 -> AG wire bytes halved, same PE
    rate (1 cyc/row), ~2e-4 rel precision.
  - 3 eager per-layer AllGathers per step (issued right after each
    cell) instead of one combined AG after L3 -> each has ~a full step
    of compute to hide under.
  - Projection + softmax sharded by time (core j handles t % 8 == j)
    instead of replicated 8x: runs every 8th step from a fp16 h3
    history (AG3 gathers straight into hist3[t]), read back with a
    partition-id-dependent dynamic DMA offset. Host re-interleaves.
"""
import numpy as np

import concourse.bass as bass
import concourse.bacc as bacc
import concourse.mybir as mybir
from concourse.tile import TileContext
from concourse.bass_utils import run_bass_kernel_spmd

F32 = mybir.dt.float32
F32R = mybir.dt.float32r
F16 = mybir.dt.float16
ACT = mybir.ActivationFunctionType
ALU = mybir.AluOpType

B, T, U, V, L, F = 64, 256, 1024, 512, 512, 64
NCORE = 8
GS = 512          # gates per core (4 x 128 units)
US = 128          # units per core
KT = U // 128     # 8 contract tiles over hidden
TWIN = T // NCORE
XCH = 32          # x chunk steps


def build():
    nc = bacc.Bacc("TRN2", target_bir_lowering=False)
    TT = T

    # ---- external inputs (per-core prepped content)
    latT = nc.dram_tensor("latT", [L, B], F32R, kind="ExternalInput")
    xT = nc.dram_tensor("xT", [F, TT, B], F32R, kind="ExternalInput")
    k1L = nc.dram_tensor("k1L", [L, GS], F32R, kind="ExternalInput")
    k1x = nc.dram_tensor("k1x", [F, GS], F32R, kind="ExternalInput")
    rk1 = nc.dram_tensor("rk1", [U, GS], F16, kind="ExternalInput")
    k2 = nc.dram_tensor("k2", [U, GS], F16, kind="ExternalInput")
    rk2 = nc.dram_tensor("rk2", [U, GS], F16, kind="ExternalInput")
    k3 = nc.dram_tensor("k3", [U, GS], F16, kind="ExternalInput")
    rk3 = nc.dram_tensor("rk3", [U, GS], F16, kind="ExternalInput")
    wp = nc.dram_tensor("wp", [U, V], F16, kind="ExternalInput")
    b1r = nc.dram_tensor("b1r", [1, GS], F32R, kind="ExternalInput")
    b2r = nc.dram_tensor("b2r", [1, GS], F32R, kind="ExternalInput")
    b3r = nc.dram_tensor("b3r", [1, GS], F32R, kind="ExternalInput")
    bvr = nc.dram_tensor("bvr", [1, V], F32R, kind="ExternalInput")
    eye64 = nc.dram_tensor("eye64", [B, B], F32R, kind="ExternalInput")
    eye64h = nc.dram_tensor("eye64h", [B, B], F16, kind="ExternalInput")
    ones1 = nc.dram_tensor("ones1", [1, B], F32R, kind="ExternalInput")

    y_out = nc.dram_tensor("y_out", [B, TWIN, V], F32, kind="ExternalOutput")
    l_out = nc.dram_tensor("l_out", [B, TWIN, V], F32, kind="ExternalOutput")

    # ---- internal DRAM
    din = [[nc.dram_tensor(f"din{l}_{p}", [128, B], F16) for p in range(2)]
           for l in range(3)]
    dout = [[nc.dram_tensor(f"dout{l}_{p}", [KT, 128, B], F16,
                            addr_space="Shared") for p in range(2)]
            for l in range(2)]
    hist3 = nc.dram_tensor("hist3", [TT, KT, 128, B], F16,
                           addr_space="Shared")
    RG = [list(range(NCORE))]

    with TileContext(nc) as tc:
        with (
            tc.tile_pool(name="wpool", bufs=1) as wpool,
            tc.tile_pool(name="state", bufs=1) as state,
            tc.tile_pool(name="work", bufs=2) as work,
            tc.tile_pool(name="psg", bufs=2, space="PSUM") as psg,
            tc.tile_pool(name="psp", bufs=1, space="PSUM") as psp,
            tc.tile_pool(name="pstr", bufs=1, space="PSUM") as pstr,
        ):
            # ---- resident weights
            def load_w(name, dram, kt, ncol, dt):
                t = wpool.tile([128, kt, ncol], dt, name=name, tag=name)
                nc.sync.dma_start(
                    t[:, :, :], dram.ap().rearrange("(k p) g -> p k g", p=128))
                return t

            rk1s = load_w("rk1s", rk1, KT, GS, F16)
            rk2s = load_w("rk2s", rk2, KT, GS, F16)
            rk3s = load_w("rk3s", rk3, KT, GS, F16)
            k2s = load_w("k2s", k2, KT, GS, F16)
            k3s = load_w("k3s", k3, KT, GS, F16)
            ws = load_w("ws", wp, KT, V, F16)
            k1Ls = load_w("k1Ls", k1L, L // 128, GS, F32R)
            latTs = load_w("latTs", latT, L // 128, B, F32R)
            k1xs = wpool.tile([F, GS], F32R, tag="k1xs")
            nc.sync.dma_start(k1xs[:, :], k1x[:, :])
            e64 = wpool.tile([B, B], F32R, tag="e64")
            nc.sync.dma_start(e64[:, :], eye64[:, :])
            e64h = wpool.tile([B, B], F16, tag="e64h")
            nc.sync.dma_start(e64h[:, :], eye64h[:, :])
            on1 = wpool.tile([1, B], F32R, tag="on1")
            nc.sync.dma_start(on1[:, :], ones1[:, :])
            b1s = wpool.tile([1, GS], F32R, tag="b1s")
            nc.sync.dma_start(b1s[:, :], b1r[:, :])
            b2s = wpool.tile([1, GS], F32R, tag="b2s")
            nc.sync.dma_start(b2s[:, :], b2r[:, :])
            b3s = wpool.tile([1, GS], F32R, tag="b3s")
            nc.sync.dma_start(b3s[:, :], b3r[:, :])
            bvs = wpool.tile([1, V], F32R, tag="bvs")
            nc.sync.dma_start(bvs[:, :], bvr[:, :])

            pid = nc.gpsimd.partition_id()

            # ---- latentb = lat.T @ k1L + b1  (once)
            lb_ps = psp.tile([B, GS], F32, tag="pps")
            for k in range(L // 128):
                nc.tensor.matmul(lb_ps[:, :], latTs[:, k, :], k1Ls[:, k, :],
                                 start=(k == 0), stop=False)
            nc.tensor.matmul(lb_ps[:, :], on1[:, :], b1s[:, :],
                             start=False, stop=True)
            latentb = state.tile([B, GS], F32R, tag="latentb")
            nc.vector.tensor_copy(latentb[:, :], lb_ps[:, :])

            # ---- states
            cst = [state.tile([B, US], F32, name=f"c{l}", tag=f"c{l}")
                   for l in range(3)]
            for c in cst:
                nc.vector.memset(c[:, :], 0.0)
            # gathered h (transposed, fp16): hp[l][par] = [128, KT, B]
            hp = [[state.tile([128, KT, B], F16, name=f"hp{l}_{p}",
                              tag=f"hp{l}_{p}") for p in range(2)]
                  for l in range(3)]
            for row in hp:
                for t_ in row:
                    nc.vector.memset(t_[:, :, :], 0.0)
            evac = [[state.tile([128, B], F16, name=f"ev{l}_{p}",
                                tag=f"ev{l}_{p}") for p in range(2)]
                    for l in range(3)]
            for row in evac:
                for t_ in row:
                    nc.vector.memset(t_[:, :], 0.0)

            xsb = [state.tile([F, XCH, B], F32R, name=f"xsb{p}", tag=f"xsb{p}")
                   for p in range(2)]
            pstage = state.tile([128, KT, B], F16, tag="pstage")

            def cell(layer, s, rks, ins_fn):
                par = s % 2
                g = psg.tile([B, GS], F32, name=f"g{layer}_{s}",
                             tag=f"g{layer}")
                ins_fn(g)
                hpl = hp[layer][(s + 1) % 2]
                for k in range(KT):
                    nc.tensor.matmul(g[:, :], hpl[:, k, :], rks[:, k, :],
                                     start=False, stop=(k == KT - 1))
                gs = work.tile([B, GS], F32, name=f"gs{layer}_{s}",
                               tag=f"gs{layer}")
                nc.scalar.activation(gs[:, 0:384], g[:, 0:384], ACT.Sigmoid)
                nc.scalar.activation(gs[:, 384:512], g[:, 384:512], ACT.Tanh)
                c = cst[layer]
                tmp = work.tile([B, US], F32, name=f"tmp{layer}_{s}",
                                tag=f"tmp{layer}")
                nc.vector.tensor_tensor(tmp[:, :], gs[:, 0:128],
                                        gs[:, 384:512], ALU.mult)
                nc.vector.tensor_tensor(c[:, :], gs[:, 128:256], c[:, :],
                                        ALU.mult)
                nc.vector.tensor_tensor(c[:, :], c[:, :], tmp[:, :], ALU.add)
                th = work.tile([B, US], F32, name=f"th{layer}_{s}",
                               tag=f"th{layer}")
                nc.scalar.activation(th[:, :], c[:, :], ACT.Tanh)
                hsb = work.tile([B, US], F16, name=f"h{layer}_{s}",
                                tag=f"h{layer}")
                nc.vector.tensor_tensor(hsb[:, :], gs[:, 256:384], th[:, :],
                                        ALU.mult)
                # transpose -> evac slot (fp16)
                tp = pstr.tile([128, B], F16, name=f"tp{layer}_{s}",
                               tag="trps")
                nc.tensor.transpose(tp[:, :], hsb[:, :], e64h[:, :])
                nc.vector.tensor_copy(evac[layer][par][:, :], tp[:, :])

            def comm(layer, s, t):
                """AllGather layer's h(t) slice; scatter into hp / hist3."""
                par = s % 2
                nc.sync.dma_start(din[layer][par].ap(), evac[layer][par][:, :])
                if layer < 2:
                    out_ap = dout[layer][par].ap()
                    scat = out_ap.rearrange("k p b -> p k b")
                else:
                    out_ap = hist3[t:t + 1, :, :, :]  # h3(t) -> hist3[t]
                    scat = out_ap.rearrange("o k p b -> p (o k) b")
                nc.gpsimd.collective_compute(
                    "AllGather", ALU.bypass, replica_groups=RG,
                    ins=[din[layer][par].ap().opt()],
                    outs=[out_ap.opt()],
                )
                nc.sync.dma_start(hp[layer][par][:, :, :], scat)

            nsteps = TT + 3
            for s in range(nsteps):
                t1, t2, t3 = s, s - 1, s - 2

                # ---- L1
                if t1 < TT:
                    if t1 % XCH == 0:
                        xb = xsb[(t1 // XCH) % 2]
                        hi = min(XCH, TT - t1)
                        nc.sync.dma_start(xb[:, 0:hi, :], xT[:, t1:t1 + hi, :])

                    def ins1(g, t1=t1):
                        xb = xsb[(t1 // XCH) % 2]
                        nc.tensor.matmul(g[:, :], xb[:, t1 % XCH, :],
                                         k1xs[:, :], start=True, stop=False)
                        nc.tensor.matmul(g[:, :], e64[:, :], latentb[:, :],
                                         start=False, stop=False)
                    cell(0, s, rk1s, ins1)
                    comm(0, s, t1)

                # ---- L2
                if 0 <= t2 < TT:
                    def ins2(g, s=s):
                        nc.tensor.matmul(g[:, :], on1[:, :], b2s[:, :],
                                         start=True, stop=False)
                        hp1 = hp[0][(s + 1) % 2]
                        for k in range(KT):
                            nc.tensor.matmul(g[:, :], hp1[:, k, :],
                                             k2s[:, k, :], start=False,
                                             stop=False)
                    cell(1, s, rk2s, ins2)
                    comm(1, s, t2)

                # ---- L3
                if 0 <= t3 < TT:
                    def ins3(g, s=s):
                        nc.tensor.matmul(g[:, :], on1[:, :], b3s[:, :],
                                         start=True, stop=False)
                        hp2 = hp[1][(s + 1) % 2]
                        for k in range(KT):
                            nc.tensor.matmul(g[:, :], hp2[:, k, :],
                                             k3s[:, k, :], start=False,
                                             stop=False)
                    cell(2, s, rk3s, ins3)
                    comm(2, s, t3)

                # ---- projection + softmax, time-sharded: core j does
                # times {8m + j}; group m runs at step s = 8m + 9 when
                # h3(8m..8m+7) are all in hist3.
                if s >= 9 and (s - 9) % 8 == 0 and (s - 9) // 8 < TWIN:
                    m = (s - 9) // 8
                    nc.gpsimd.dma_start(
                        pstage[:, :, :],
                        hist3[bass.ds(8 * m + pid, 1), :, :, :].rearrange(
                            "o k p b -> p (o k) b"),
                    )
                    pps = psp.tile([B, V], F32, name=f"pps{m}", tag="pps")
                    nc.tensor.matmul(pps[:, :], on1[:, :], bvs[:, :],
                                     start=True, stop=False)
                    for k in range(KT):
                        nc.tensor.matmul(pps[:, :], pstage[:, k, :],
                                         ws[:, k, :], start=False,
                                         stop=(k == KT - 1))
                    lsb = work.tile([B, V], F32, name=f"lsb{m}", tag="lsb")
                    nc.vector.tensor_copy(lsb[:, :], pps[:, :])
                    nmx = work.tile([B, 1], F32, name=f"nmx{m}", tag="nmx")
                    nc.vector.tensor_reduce(nmx[:, :], lsb[:, :],
                                            axis=mybir.AxisListType.X,
                                            op=ALU.max, negate=True)
                    esb = work.tile([B, V], F32, name=f"esb{m}", tag="esb")
                    sm = work.tile([B, 1], F32, name=f"sm{m}", tag="sm")
                    nc.scalar.activation(esb[:, :], lsb[:, :], ACT.Exp,
                                         bias=nmx[:, 0:1], accum_out=sm[:, 0:1])
                    rs = work.tile([B, 1], F32, name=f"rs{m}", tag="rs")
                    nc.vector.reciprocal(rs[:, :], sm[:, :])
                    ysb = work.tile([B, V], F32, name=f"ysb{m}", tag="ysb")
                    nc.vector.tensor_scalar(ysb[:, :], esb[:, :], rs[:, 0:1],
                                            None, ALU.mult)
                    nc.sync.dma_start(y_out[:, m:m + 1, :], ysb[:, :])
                    nc.sync.dma_start(l_out[:, m:m + 1, :], lsb[:, :])

    nc.compile()
    return nc


_built = {}


def _get_nc():
    if "nc" not in _built:
        _built["nc"] = build()
    return _built["nc"]


def _prep_inputs(latent, x, k1, rk1, b1, k2, rk2, b2, k3, rk3, b3, w, b):
    f32, f16 = np.float32, np.float16
    latent = np.asarray(latent, f32)
    x = np.asarray(x, f32)
    in_maps = []
    latT = np.ascontiguousarray(latent.T)                   # [L, B]
    xT = np.ascontiguousarray(np.transpose(x, (2, 1, 0)))   # [F, T, B]
    eye64v = np.eye(B, dtype=f32)
    eye64hv = np.eye(B, dtype=f16)
    ones1v = np.ones((1, B), f32)
    for j in range(NCORE):
        u0 = j * US
        cols = np.concatenate([
            np.arange(u0, u0 + US),                  # i
            np.arange(U + u0, U + u0 + US),          # f
            np.arange(3 * U + u0, 3 * U + u0 + US),  # o
            np.arange(2 * U + u0, 2 * U + u0 + US),  # g
        ])
        in_maps.append({
            "latT": latT,
            "xT": xT,
            "k1L": np.ascontiguousarray(k1[:L, cols]),
            "k1x": np.ascontiguousarray(k1[L:, cols]),
            "rk1": np.ascontiguousarray(np.asarray(rk1, f16)[:, cols]),
            "k2": np.ascontiguousarray(np.asarray(k2, f16)[:, cols]),
            "rk2": np.ascontiguousarray(np.asarray(rk2, f16)[:, cols]),
            "k3": np.ascontiguousarray(np.asarray(k3, f16)[:, cols]),
            "rk3": np.ascontiguousarray(np.asarray(rk3, f16)[:, cols]),
            "wp": np.ascontiguousarray(np.asarray(w, f16)),
            "b1r": np.ascontiguousarray(np.asarray(b1, f32)[cols][None, :]),
            "b2r": np.ascontiguousarray(np.asarray(b2, f32)[cols][None, :]),
            "b3r": np.ascontiguousarray(np.asarray(b3, f32)[cols][None, :]),
            "bvr": np.ascontiguousarray(np.asarray(b, f32)[None, :]),
            "eye64": eye64v,
            "eye64h": eye64hv,
            "ones1": ones1v,
        })
    return in_maps


def run(**inputs):
    nc = _get_nc()
    in_maps = _prep_inputs(**inputs)
    res = run_bass_kernel_spmd(nc, in_maps, core_ids=list(range(NCORE)))
    y = np.zeros((B, T, V), np.float32)
    lg = np.zeros((B, T, V), np.float32)
    for j in range(NCORE):
        y[:, j::8, :] = res.results[j]["y_out"]
        lg[:, j::8, :] = res.results[j]["l_out"]
    return y, lg


def kernel(**inputs):
    return run(**inputs)
